# revision 1
# baseline (speedup 1.0000x reference)
"""DecoderLSTM Trainium2 kernel — 8-core data-parallel over batch.

Problem: 2-layer LSTM (H=512, B=512, T=128) where the step input is the sum of
the two layers' hidden states, followed by a 3-layer MLP head applied to the
[B, T, H] hidden-sum sequence.

Strategy (per core, B_c = 64 batch rows, zero collectives):
  - LSTM gates computed as g[B_c, 4H] with the *activations* stationary on the
    PE array ([K=128, M=64] tiles of x^T / h^T) and the *weights* streaming as
    the moving operand in fp32r (full-rate, ~1.5e-4 numerics) 512-col chunks.
  - h_new is transposed back to [H, B_c] each step with PE transpose-mode
    matmuls so the next step's stationary operands need no extra work.
  - Hidden sums are staged transposed in SBUF rings and flushed to a DRAM
    scratch every 8 steps; the MLP head then runs chunk-wise (512 rows at a
    time): fc1/fc2 weights-stationary with fused bias+ReLU on the scalar
    engine, fc3 activations-stationary so the result lands in [rows, H] layout
    for direct output DMA.
  - Raw bass (no Tile): explicit per-engine programs and semaphores.
"""

import ml_dtypes
import numpy as np

import concourse.bass as bass
import concourse.mybir as mybir
from concourse.bass_utils import run_bass_kernel_spmd

F32 = mybir.dt.float32
F32R = mybir.dt.float32r
BF16 = mybir.dt.bfloat16
AF = mybir.ActivationFunctionType
MUL = mybir.AluOpType.mult
ADD = mybir.AluOpType.add

NCORES = 8
B, H, T, L = 512, 512, 128, 2
BC = B // NCORES          # 64 batch rows per core
G = 4 * H                 # 2048 gate rows
KT = H // 128             # 4 K-tiles
NCH = 4                   # gate column chunks of 512
RING = 8                  # steps per outs ring flush
NFLUSH = T // RING        # 16
NCHUNK = (BC * T) // 512  # 16 MLP row chunks of 512

# ---- semaphore value schedules (pure functions of step/chunk) ----------------
# dve events per LSTM step: c0, h0, hT0, c1, h1, hT1, hsum  (1-based)
_DVE_E = {"c0": 1, "h0": 2, "hT0": 3, "c1": 4, "h1": 5, "hT1": 6, "hsum": 7}
# act events per LSTM step: sig0, tang0, tanc0, sig1, tang1, tanc1
_ACT_E = {"sig0": 1, "tang0": 2, "tanc0": 3, "sig1": 4, "tang1": 5, "tanc1": 6}
# pe events per LSTM step: l0c012, l0c3, l0T, l1c012, l1c3, l1T
_PE_E = {"l0c012": 1, "l0c3": 2, "l0T": 3, "l1c012": 4, "l1c3": 5, "l1T": 6}


def dve_v(t, e):
    return 1 + 7 * t + _DVE_E[e]          # +1 for initial xT copy


def act_v(t, e):
    return 6 * t + _ACT_E[e]


def pe_v(t, e):
    return 1 + 6 * t + _PE_E[e]           # +1 for x transposes


DVE_LSTM_END = 1 + 7 * T
ACT_LSTM_END = 6 * T
PE_LSTM_END = 1 + 6 * T


def dve_mlp(j, m):                        # after o3 copy (j, rowm m)
    return DVE_LSTM_END + 4 * j + m + 1


def act_mlp(j, which):                    # which: 1 = relu1, 2 = relu2
    return ACT_LSTM_END + 2 * j + which


# pe events per MLP chunk: fc1, fc2, fc3m0..fc3m3
def pe_mlp(j, e):
    base = PE_LSTM_END + 6 * j
    if e == "fc1":
        return base + 1
    if e == "fc2":
        return base + 2
    return base + 3 + int(e)


NPRE = 16                                 # preload DMA count
DMA_IN_PRE = 16 * NPRE

# per-rep semaphore totals (for benchmark builds that loop the whole program)
PE_TOT = PE_LSTM_END + 6 * NCHUNK
ACT_TOT = ACT_LSTM_END + 2 * NCHUNK
DVE_TOT = DVE_LSTM_END + 4 * NCHUNK
DMA_IN_TOT = 16 * (NPRE + NCHUNK)
DMA_OUT_TOT = 16 * (NFLUSH + 4 * NCHUNK)


def dma_in_load(j):                       # after MLP chunk-j actT load
    return DMA_IN_PRE + 16 * (j + 1)


def dma_out_flush(f):
    return 16 * (f + 1)


def dma_out_mlp(j, m):
    return 16 * (NFLUSH + 4 * j + m + 1)


def build_nc(reps=1, t_steps=None, mlp=True):
    global T, NFLUSH, NCHUNK, DVE_LSTM_END, ACT_LSTM_END, PE_LSTM_END
    global PE_TOT, ACT_TOT, DVE_TOT, DMA_IN_TOT, DMA_OUT_TOT
    if t_steps is not None:
        T = t_steps
        NFLUSH = T // RING
        NCHUNK = (BC * T) // 512
        DVE_LSTM_END = 1 + 7 * T
        ACT_LSTM_END = 6 * T
        PE_LSTM_END = 1 + 6 * T
        PE_TOT = PE_LSTM_END + 6 * NCHUNK
        ACT_TOT = ACT_LSTM_END + 2 * NCHUNK
        DVE_TOT = DVE_LSTM_END + 4 * NCHUNK
        DMA_IN_TOT = 16 * (NPRE + NCHUNK)
        DMA_OUT_TOT = 16 * (NFLUSH + 4 * NCHUNK)
    nc = bass.Bass("TRN2", target_bir_lowering=False, debug=False,
                   num_devices=NCORES)

    # ---- DRAM I/O ----
    x_d = nc.dram_tensor("x", [BC, H], F32, kind="ExternalInput")
    wih_d = [nc.dram_tensor(f"wih{l}", [KT, 128, G], F32, kind="ExternalInput") for l in range(L)]
    whh_d = [nc.dram_tensor(f"whh{l}", [KT, 128, G], F32, kind="ExternalInput") for l in range(L)]
    bias_d = [nc.dram_tensor(f"bias{l}", [1, G], BF16, kind="ExternalInput") for l in range(L)]
    fc1w_d = nc.dram_tensor("fc1w", [KT, 128, 512], F32, kind="ExternalInput")
    fc2w_d = nc.dram_tensor("fc2w", [KT, 128, 512], F32, kind="ExternalInput")
    fc3w_d = nc.dram_tensor("fc3w", [KT, 128, 512], F32, kind="ExternalInput")
    fc1b_d = nc.dram_tensor("fc1b", [128, 4], F32, kind="ExternalInput")
    fc2b_d = nc.dram_tensor("fc2b", [128, 4], F32, kind="ExternalInput")
    fc3b_d = nc.dram_tensor("fc3b", [1, 512], F32, kind="ExternalInput")
    ones64_d = nc.dram_tensor("ones64", [1, 64], BF16, kind="ExternalInput")
    ones128_d = nc.dram_tensor("ones128", [1, 128], F32, kind="ExternalInput")
    id64_d = nc.dram_tensor("id64", [64, 64], F32, kind="ExternalInput")
    outsT_d = nc.dram_tensor("outsT", [KT, 128, T, BC], F32, kind="Internal")
    out_d = nc.dram_tensor("out", [BC, T, H], F32, kind="ExternalOutput")

    # ---- hand-drawn SBUF map (per-partition byte offsets) ----
    off = [(nc.sbuf_base + 63) // 64 * 64]

    def at(name, shape, dtype, align=32, offset=None):
        o = (off[0] + align - 1) // align * align if offset is None else offset
        h = nc.alloc_sbuf_tensor_at(name, shape, dtype, offset=o)
        sz = int(np.prod(shape[1:])) * mybir.dt.size(dtype)
        if offset is None:
            off[0] = o + sz
        return h

    wih = [at(f"wih{l}s", [128, KT, G], F32R) for l in range(L)]
    whh = [at(f"whh{l}s", [128, KT, G], F32R) for l in range(L)]
    fc1w = at("fc1ws", [128, KT, 512], F32R)
    fc2w = at("fc2ws", [128, KT, 512], F32R)
    fc3w = at("fc3ws", [128, KT, 512], F32R)
    biasr = [at(f"bias{l}s", [1, G], BF16) for l in range(L)]
    fc3br = at("fc3bs", [1, 512], F32R)
    ones64 = at("ones64s", [1, 64], BF16)
    ones128 = at("ones128s", [1, 128], F32R)
    id64 = at("id64s", [64, 64], F32)
    fc1b = at("fc1bs", [128, 4], F32)
    fc2b = at("fc2bs", [128, 4], F32)

    lstm_base = off[0]
    ring = [at(f"ring{r}", [128, KT, RING, BC], F32) for r in range(2)]
    sig = at("sig", [64, 1536], F32)
    tang = at("tang", [64, 512], F32)
    tanc = at("tanc", [64, 512], F32)
    hnew = at("hnew", [64, 512], F32)
    tmp = at("tmp", [64, 512], F32)
    c_sb = [at(f"c{l}", [64, 512], F32) for l in range(L)]
    hsumT = at("hsumT", [128, KT, BC], F32R)
    h0T = at("h0T", [128, KT, BC], F32R)
    h1T = at("h1T", [128, KT, BC], F32R)
    # x_sb and xT are dead after step 0 starts: alias onto ring0 / hsumT
    rng0_off = ring[0].manual_sbuf_range[0]
    x_sb = at("x_sb", [64, 512], F32, offset=rng0_off)
    xT = at("xT", [128, KT, BC], F32R, offset=hsumT.manual_sbuf_range[0])
    assert off[0] <= nc.SBUF_PARTITION_SIZE_BYTES, off[0]

    # MLP working set aliases the LSTM working region (used strictly after it)
    off[0] = lstm_base
    actT = [at(f"actT{r}", [128, KT, 512], F32R) for r in range(2)]
    out1 = at("out1", [128, KT, 512], F32R)
    out2 = at("out2", [128, KT, 512], F32R)
    out3 = [at(f"out3_{m}", [128, 512], F32) for m in range(4)]
    assert off[0] <= nc.SBUF_PARTITION_SIZE_BYTES, off[0]

    with (
        nc.psum_tensor("P", [128, 4096], F32) as P,
        nc.semaphore("dma_in") as dma_in,
        nc.semaphore("dma_out") as dma_out,
        nc.semaphore("pe_s") as pe_s,
        nc.semaphore("act_s") as act_s,
        nc.semaphore("dve_s") as dve_s,
        nc.Block() as block,
    ):
        Pap = P.ap()
        G0 = Pap[0:64, 0:2048]
        G1 = Pap[0:64, 2048:4096]
        Tp = [Pap[0:128, 0:256], Pap[0:128, 2048:2304]]

        def stat_x(l, t):
            """stationary tiles ([128, BC] k-tiles) for the layer-l input."""
            if l == 1:
                return h0T
            return xT if t == 0 else hsumT

        # ---------------- SYNC: all DMA ----------------
        @block.sync
        def _(sync):
          for rep in range(reps):
            oD, oO, oP = rep * DVE_TOT, rep * DMA_OUT_TOT, rep * PE_TOT
            if rep > 0:
                sync.wait_ge(dma_out, rep * DMA_OUT_TOT)

            def load(dst_ap, src_ap):
                sync.dma_start(out=dst_ap, in_=src_ap).then_inc(dma_in, 16)

            load(x_sb.ap(), x_d.ap())
            for l in range(L):
                load(wih[l].ap(), wih_d[l].ap().bitcast(F32R).rearrange("k p c -> p k c"))
                load(whh[l].ap(), whh_d[l].ap().bitcast(F32R).rearrange("k p c -> p k c"))
                load(biasr[l].ap(), bias_d[l].ap())
            load(fc1w.ap(), fc1w_d.ap().bitcast(F32R).rearrange("k p c -> p k c"))
            load(fc2w.ap(), fc2w_d.ap().bitcast(F32R).rearrange("k p c -> p k c"))
            load(fc3w.ap(), fc3w_d.ap().bitcast(F32R).rearrange("k p c -> p k c"))
            load(fc3br.ap(), fc3b_d.ap().bitcast(F32R))
            load(ones64.ap(), ones64_d.ap())
            load(ones128.ap(), ones128_d.ap().bitcast(F32R))
            load(id64.ap(), id64_d.ap())
            load(fc1b.ap(), fc1b_d.ap())
            load(fc2b.ap(), fc2b_d.ap())

            # ring flushes
            for f in range(NFLUSH):
                sync.wait_ge(dve_s, oD + dve_v(RING * f + RING - 1, "hsum"))
                sync.dma_start(
                    out=outsT_d.ap()[:, :, RING * f:RING * (f + 1), :]
                        .rearrange("k p t b -> p k t b"),
                    in_=ring[f % 2].ap(),
                ).then_inc(dma_out, 16)

            # MLP: prefetch loads + outputs
            def mload(j):
                sync.wait_ge(dma_out, oO + 16 * NFLUSH)
                if j >= 2:
                    sync.wait_ge(pe_s, oP + pe_mlp(j - 2, "fc1"))
                sync.dma_start(
                    out=actT[j % 2].ap(),
                    in_=outsT_d.ap()[:, :, RING * j:RING * (j + 1), :].bitcast(F32R)
                        .rearrange("k p t b -> p k (t b)"),
                ).then_inc(dma_in, 16)

            mload(0)
            mload(1)
            for j in range(NCHUNK):
                for m in range(4):
                    sync.wait_ge(dve_s, oD + dve_mlp(j, m))
                    tt = 8 * j + 2 * m
                    sync.dma_start(
                        out=out_d.ap()[:, tt:tt + 2, :].rearrange("b u h -> u b h"),
                        in_=out3[m].ap(),
                    ).then_inc(dma_out, 16)
                if j + 2 < NCHUNK:
                    mload(j + 2)

        # ---------------- PE ----------------
        @block.tensor
        def _(tensor):
          for rep in range(reps):
            oI, oD, oA = rep * DMA_IN_TOT, rep * DVE_TOT, rep * ACT_TOT

            def bias_h_group(l, t, gp):
                """bias + recurrent-part matmuls for all 4 chunks of layer l."""
                hstat = h0T if l == 0 else h1T
                for c in range(NCH):
                    cs = slice(512 * c, 512 * (c + 1))
                    tensor.matmul(gp[:, cs], ones64.ap(), biasr[l].ap()[:, cs],
                                  start=True, stop=False)
                    if t > 0:
                        for k in range(KT):
                            tensor.matmul(gp[:, cs], hstat.ap()[:, k, :],
                                          whh[l].ap()[:, k, cs],
                                          start=False, stop=False)

            def x_group(l, t, gp):
                xstat = stat_x(l, t)
                for c in range(NCH):
                    cs = slice(512 * c, 512 * (c + 1))
                    for k in range(KT):
                        mm = tensor.matmul(gp[:, cs], xstat.ap()[:, k, :],
                                           wih[l].ap()[:, k, cs],
                                           start=False, stop=(k == KT - 1))
                    if c == 2:
                        mm.then_inc(pe_s, 1)
                mm.then_inc(pe_s, 1)

            def transpose_h(l):
                for c in range(4):
                    mm = tensor.transpose(Tp[l][:, 64 * c:64 * (c + 1)],
                                          hnew.ap()[:, 128 * c:128 * (c + 1)],
                                          id64.ap())
                mm.then_inc(pe_s, 1)

            # prologue: transpose x
            tensor.wait_ge(dma_in, oI + DMA_IN_PRE)
            for c in range(4):
                mm = tensor.transpose(Tp[0][:, 64 * c:64 * (c + 1)],
                                      x_sb.ap()[:, 128 * c:128 * (c + 1)],
                                      id64.ap())
            mm.then_inc(pe_s, 1)

            for t in range(T):
                # L0 x-part (stationary: xT at t=0 else hsumT(t-1))
                if t == 0:
                    # bias group must come after the xT copy frees Tp[0]
                    tensor.wait_ge(dve_s, oD + 1)
                    bias_h_group(0, 0, G0)
                else:
                    tensor.wait_ge(dve_s, oD + dve_v(t - 1, "hsum"))
                x_group(0, t, G0)
                # L1 bias + recurrent part
                if t > 0:
                    tensor.wait_ge(act_s, oA + act_v(t - 1, "tang1"))
                bias_h_group(1, t, G1)
                # transpose h0
                tensor.wait_ge(dve_s, oD + dve_v(t, "h0"))
                transpose_h(0)
                # L1 x-part (stationary: h0T(t))
                tensor.wait_ge(dve_s, oD + dve_v(t, "hT0"))
                x_group(1, t, G1)
                # transpose h1
                tensor.wait_ge(dve_s, oD + dve_v(t, "h1"))
                transpose_h(1)
                # next step L0 bias + h-part
                if t + 1 < T:
                    tensor.wait_ge(act_s, oA + act_v(t, "tang0"))
                    bias_h_group(0, t + 1, G0)

            # ---- MLP ----
            ps1 = [Pap[:, 512 * m:512 * (m + 1)] for m in range(4)]
            ps2 = [Pap[:, 2048 + 512 * m:2048 + 512 * (m + 1)] for m in range(4)]
            for j in range(NCHUNK):
                tensor.wait_ge(dma_in, oI + dma_in_load(j))
                if j >= 1:
                    tensor.wait_ge(dve_s, oD + dve_mlp(j - 1, 3))
                a = actT[j % 2]
                for m in range(4):
                    for k in range(KT):
                        mm = tensor.matmul(ps1[m], fc1w.ap()[:, k, 128 * m:128 * (m + 1)],
                                           a.ap()[:, k, :],
                                           start=(k == 0), stop=(k == KT - 1))
                mm.then_inc(pe_s, 1)
                tensor.wait_ge(act_s, oA + act_mlp(j, 1))
                for m in range(4):
                    for k in range(KT):
                        mm = tensor.matmul(ps2[m], fc2w.ap()[:, k, 128 * m:128 * (m + 1)],
                                           out1.ap()[:, k, :],
                                           start=(k == 0), stop=(k == KT - 1))
                mm.then_inc(pe_s, 1)
                tensor.wait_ge(act_s, oA + act_mlp(j, 2))
                for m in range(4):
                    tensor.matmul(ps1[m], ones128.ap(), fc3br.ap(),
                                  start=True, stop=False)
                    for k in range(KT):
                        mm = tensor.matmul(ps1[m],
                                           out2.ap()[:, k, 128 * m:128 * (m + 1)],
                                           fc3w.ap()[:, k, :],
                                           start=False, stop=(k == KT - 1))
                    mm.then_inc(pe_s, 1)

        # ---------------- ACT (scalar) ----------------
        @block.scalar
        def _(scalar):
          for rep in range(reps):
            oP, oD = rep * PE_TOT, rep * DVE_TOT
            for t in range(T):
                for l in range(L):
                    gp = G0 if l == 0 else G1
                    scalar.wait_ge(pe_s, oP + pe_v(t, f"l{l}c012"))
                    scalar.activation(sig.ap(), gp[:, 0:1536], AF.Sigmoid
                                      ).then_inc(act_s, 1)
                    scalar.wait_ge(pe_s, oP + pe_v(t, f"l{l}c3"))
                    scalar.activation(tang.ap(), gp[:, 1536:2048], AF.Tanh
                                      ).then_inc(act_s, 1)
                    scalar.wait_ge(dve_s, oD + dve_v(t, f"c{l}"))
                    scalar.activation(tanc.ap(), c_sb[l].ap(), AF.Tanh
                                      ).then_inc(act_s, 1)
            # MLP relu with fused per-partition bias
            for j in range(NCHUNK):
                scalar.wait_ge(pe_s, oP + pe_mlp(j, "fc1"))
                for m in range(4):
                    a = scalar.activation(out1.ap()[:, m, :],
                                          Pap[:, 512 * m:512 * (m + 1)], AF.Relu,
                                          bias=fc1b.ap()[:, m:m + 1])
                a.then_inc(act_s, 1)
                scalar.wait_ge(pe_s, oP + pe_mlp(j, "fc2"))
                for m in range(4):
                    a = scalar.activation(out2.ap()[:, m, :],
                                          Pap[:, 2048 + 512 * m:2048 + 512 * (m + 1)],
                                          AF.Relu, bias=fc2b.ap()[:, m:m + 1])
                a.then_inc(act_s, 1)

        # ---------------- DVE (vector) ----------------
        @block.vector
        def _(vector):
          for rep in range(reps):
            oP, oA, oO = rep * PE_TOT, rep * ACT_TOT, rep * DMA_OUT_TOT
            vector.wait_ge(pe_s, oP + 1)
            vector.tensor_copy(xT.ap().rearrange("p k b -> p (k b)"), Tp[0]
                               ).then_inc(dve_s, 1)
            for t in range(T):
                for l in range(L):
                    vector.wait_ge(act_s, oA + act_v(t, f"tang{l}"))
                    # tmp = i * tanh(g)
                    vector.tensor_tensor(tmp.ap(), sig.ap()[:, 0:512], tang.ap(), MUL)
                    if t == 0:
                        vector.tensor_copy(c_sb[l].ap(), tmp.ap()).then_inc(dve_s, 1)
                    else:
                        # c = c*f + tmp
                        vector.tensor_tensor(c_sb[l].ap(), c_sb[l].ap(),
                                             sig.ap()[:, 512:1024], MUL)
                        vector.tensor_tensor(c_sb[l].ap(), c_sb[l].ap(), tmp.ap(),
                                             ADD).then_inc(dve_s, 1)
                    vector.wait_ge(act_s, oA + act_v(t, f"tanc{l}"))
                    vector.tensor_tensor(hnew.ap(), sig.ap()[:, 1024:1536],
                                         tanc.ap(), MUL).then_inc(dve_s, 1)
                    vector.wait_ge(pe_s, oP + pe_v(t, f"l{l}T"))
                    hT = h0T if l == 0 else h1T
                    vector.tensor_copy(hT.ap().rearrange("p k b -> p (k b)"), Tp[l]
                                       ).then_inc(dve_s, 1)
                # hsum + ring write
                vector.tensor_tensor(hsumT.ap(), h0T.ap(), h1T.ap(), ADD)
                blk = t // RING
                if blk >= 2:
                    vector.wait_ge(dma_out, oO + 16 * (blk - 1))
                vector.tensor_copy(ring[blk % 2].ap()[:, :, t % RING, :],
                                   hsumT.ap()).then_inc(dve_s, 1)
            # MLP psum3 -> out3 copies
            for j in range(NCHUNK):
                for m in range(4):
                    vector.wait_ge(pe_s, oP + pe_mlp(j, m))
                    if j >= 1:
                        vector.wait_ge(dma_out, oO + dma_out_mlp(j - 1, m))
                    vector.tensor_copy(out3[m].ap(), Pap[:, 512 * m:512 * (m + 1)]
                                       ).then_inc(dve_s, 1)

    return nc


_PERM = None


def _gate_perm():
    # torch gate order (i, f, g, o) -> our column order (i, f, o, g)
    global _PERM
    if _PERM is None:
        i = np.arange(512)
        _PERM = np.concatenate([i, 512 + i, 1536 + i, 1024 + i])
    return _PERM


def _prep_inputs(x, W_ih, W_hh, b_ih, b_hh, fc1_w, fc1_b, fc2_w, fc2_b, fc3_w, fc3_b):
    perm = _gate_perm()
    common = {}
    for l in range(L):
        wt = np.ascontiguousarray(W_ih[l][perm].T)          # [512, 2048]
        common[f"wih{l}"] = wt.reshape(KT, 128, G)
        wt = np.ascontiguousarray(W_hh[l][perm].T)
        common[f"whh{l}"] = wt.reshape(KT, 128, G)
        common[f"bias{l}"] = (b_ih[l] + b_hh[l])[perm].reshape(1, G).astype(ml_dtypes.bfloat16)
    common["fc1w"] = np.ascontiguousarray(fc1_w.T).reshape(KT, 128, 512)
    common["fc2w"] = np.ascontiguousarray(fc2_w.T).reshape(KT, 128, 512)
    common["fc3w"] = np.ascontiguousarray(fc3_w.T).reshape(KT, 128, 512)
    common["fc1b"] = np.ascontiguousarray(fc1_b.reshape(4, 128).T)
    common["fc2b"] = np.ascontiguousarray(fc2_b.reshape(4, 128).T)
    common["fc3b"] = fc3_b.reshape(1, 512).astype(np.float32)
    common["ones64"] = np.ones((1, 64), ml_dtypes.bfloat16)
    common["ones128"] = np.ones((1, 128), np.float32)
    common["id64"] = np.eye(64, dtype=np.float32)
    in_maps = []
    for c in range(NCORES):
        m = dict(common)
        m["x"] = np.ascontiguousarray(x[BC * c:BC * (c + 1)])
        in_maps.append(m)
    return in_maps


_NC_CACHE = None


def kernel(**inputs):
    global _NC_CACHE
    if _NC_CACHE is None:
        _NC_CACHE = build_nc()
    nc = _NC_CACHE
    in_maps = _prep_inputs(**{k: np.asarray(v) for k, v in inputs.items()})
    res = run_bass_kernel_spmd(nc, in_maps, core_ids=list(range(NCORES)))
    out = np.concatenate([res.results[c]["out"] for c in range(NCORES)], axis=0)
    return out.astype(np.float32)



# revision 6
# speedup vs baseline: 1.0777x; 1.0777x over previous
"""DecoderLSTM Trainium2 kernel — 8-core data-parallel over batch.

Problem: 2-layer LSTM (H=512, B=512, T=128) where the step input is the sum of
the two layers' hidden states, followed by a 3-layer MLP head applied to the
[B, T, H] hidden-sum sequence.

Strategy (per core, B_c = 64 batch rows, zero collectives):
  - LSTM gates computed as g[B_c, 4H] with the *activations* stationary on the
    PE array ([K=128, M=64] tiles of x^T / h^T) and the *weights* streaming as
    the moving operand in fp32r (full-rate, ~1.5e-4 numerics) 512-col chunks.
  - h_new is transposed back to [H, B_c] each step with PE transpose-mode
    matmuls so the next step's stationary operands need no extra work.
  - Hidden sums are staged transposed in SBUF rings and flushed to a DRAM
    scratch every 8 steps; the MLP head then runs chunk-wise (512 rows at a
    time): fc1/fc2 weights-stationary with fused bias+ReLU on the scalar
    engine, fc3 activations-stationary so the result lands in [rows, H] layout
    for direct output DMA.
  - Raw bass (no Tile): explicit per-engine programs and semaphores.
"""

import ml_dtypes
import numpy as np

import concourse.bass as bass
import concourse.mybir as mybir
from concourse.bass_utils import run_bass_kernel_spmd

F32 = mybir.dt.float32
F32R = mybir.dt.float32r
BF16 = mybir.dt.bfloat16
AF = mybir.ActivationFunctionType
MUL = mybir.AluOpType.mult
ADD = mybir.AluOpType.add

NCORES = 8
B, H, T, L = 512, 512, 128, 2
BC = B // NCORES          # 64 batch rows per core
G = 4 * H                 # 2048 gate rows
KT = H // 128             # 4 K-tiles
NCH = 4                   # gate column chunks of 512
RING = 8                  # steps per outs ring flush
NFLUSH = T // RING        # 16
NCHUNK = (BC * T) // 512  # 16 MLP row chunks of 512

# ---- semaphore value schedules (pure functions of step/chunk) ----------------
# dve events per LSTM step: c0, h0, hT0, c1, h1, hT1, hsum  (1-based)
_DVE_E = {"c0": 1, "h0": 2, "hT0": 3, "c1": 4, "h1": 5, "hT1": 6, "hsum": 7}
# act events per LSTM step: sig0, tang0, tanc0, sig1, tang1, tanc1
_ACT_E = {"sig0": 1, "tang0": 2, "tanc0": 3, "sig1": 4, "tang1": 5, "tanc1": 6}
# pe events per LSTM step: l0c012, l0c3, l0T, l1c012, l1c3, l1T
_PE_E = {"l0c012": 1, "l0c3": 2, "l0T": 3, "l1c012": 4, "l1c3": 5, "l1T": 6}


def dve_v(t, e):
    return 1 + 7 * t + _DVE_E[e]          # +1 for initial xT copy


def act_v(t, e):
    return 6 * t + _ACT_E[e]


def pe_v(t, e):
    return 1 + 6 * t + _PE_E[e]           # +1 for x transposes


DVE_LSTM_END = 1 + 7 * T
ACT_LSTM_END = 6 * T
PE_LSTM_END = 1 + 6 * T


def dve_mlp(j, m):                        # after o3 copy (j, rowm m)
    return DVE_LSTM_END + 4 * j + m + 1


def act_mlp(j, which):                    # which: 1 = relu1, 2 = relu2
    return ACT_LSTM_END + 2 * j + which


# pe events per MLP chunk: fc1, fc2, fc3m0..fc3m3
def pe_mlp(j, e):
    base = PE_LSTM_END + 6 * j
    if e == "fc1":
        return base + 1
    if e == "fc2":
        return base + 2
    return base + 3 + int(e)


NPRE = 16                                 # preload DMA count
DMA_IN_PRE = 16 * NPRE

# per-rep semaphore totals (for benchmark builds that loop the whole program)
PE_TOT = PE_LSTM_END + 6 * NCHUNK
ACT_TOT = ACT_LSTM_END + 2 * NCHUNK
DVE_TOT = DVE_LSTM_END + 4 * NCHUNK
DMA_IN_TOT = 16 * (NPRE + NCHUNK)
DMA_OUT_TOT = 16 * (NFLUSH + 4 * NCHUNK)


def dma_in_load(j):                       # after MLP chunk-j actT load
    return DMA_IN_PRE + 16 * (j + 1)


def dma_out_flush(f):
    return 16 * (f + 1)


def dma_out_mlp(j, m):
    return 16 * (NFLUSH + 4 * j + m + 1)


def build_nc(reps=1, t_steps=None, mlp=True):
    global T, NFLUSH, NCHUNK, DVE_LSTM_END, ACT_LSTM_END, PE_LSTM_END
    global PE_TOT, ACT_TOT, DVE_TOT, DMA_IN_TOT, DMA_OUT_TOT
    if t_steps is not None:
        T = t_steps
        NFLUSH = T // RING
        NCHUNK = (BC * T) // 512
        DVE_LSTM_END = 1 + 7 * T
        ACT_LSTM_END = 6 * T
        PE_LSTM_END = 1 + 6 * T
        PE_TOT = PE_LSTM_END + 6 * NCHUNK
        ACT_TOT = ACT_LSTM_END + 2 * NCHUNK
        DVE_TOT = DVE_LSTM_END + 4 * NCHUNK
        DMA_IN_TOT = 16 * (NPRE + NCHUNK)
        DMA_OUT_TOT = 16 * (NFLUSH + 4 * NCHUNK)
    nc = bass.Bass("TRN2", target_bir_lowering=False, debug=False,
                   num_devices=NCORES)

    # ---- DRAM I/O ----
    x_d = nc.dram_tensor("x", [BC, H], F32, kind="ExternalInput")
    wih_d = [nc.dram_tensor(f"wih{l}", [KT, 128, G], BF16, kind="ExternalInput") for l in range(L)]
    whh_d = [nc.dram_tensor(f"whh{l}", [KT, 128, G], BF16, kind="ExternalInput") for l in range(L)]
    bias_d = [nc.dram_tensor(f"bias{l}", [1, G], BF16, kind="ExternalInput") for l in range(L)]
    fc1w_d = nc.dram_tensor("fc1w", [KT, 128, 512], BF16, kind="ExternalInput")
    fc2w_d = nc.dram_tensor("fc2w", [KT, 128, 512], BF16, kind="ExternalInput")
    fc3w_d = nc.dram_tensor("fc3w", [KT, 128, 512], BF16, kind="ExternalInput")
    fc1b_d = nc.dram_tensor("fc1b", [128, 4], F32, kind="ExternalInput")
    fc2b_d = nc.dram_tensor("fc2b", [128, 4], F32, kind="ExternalInput")
    fc3b_d = nc.dram_tensor("fc3b", [1, 512], BF16, kind="ExternalInput")
    ones64_d = nc.dram_tensor("ones64", [1, 64], BF16, kind="ExternalInput")
    ones128_d = nc.dram_tensor("ones128", [1, 128], BF16, kind="ExternalInput")
    id64_d = nc.dram_tensor("id64", [64, 64], F32, kind="ExternalInput")
    outsT_d = nc.dram_tensor("outsT", [KT, 128, T, BC], BF16, kind="Internal")
    out_d = nc.dram_tensor("out", [BC, T, H], F32, kind="ExternalOutput")

    # ---- hand-drawn SBUF map (per-partition byte offsets) ----
    off = [(nc.sbuf_base + 63) // 64 * 64]

    def at(name, shape, dtype, align=32, offset=None):
        o = (off[0] + align - 1) // align * align if offset is None else offset
        h = nc.alloc_sbuf_tensor_at(name, shape, dtype, offset=o)
        sz = int(np.prod(shape[1:])) * mybir.dt.size(dtype)
        if offset is None:
            off[0] = o + sz
        return h

    wih = [at(f"wih{l}s", [128, KT, G], BF16) for l in range(L)]
    whh = [at(f"whh{l}s", [128, KT, G], BF16) for l in range(L)]
    fc1w = at("fc1ws", [128, KT, 512], BF16)
    fc2w = at("fc2ws", [128, KT, 512], BF16)
    fc3w = at("fc3ws", [128, KT, 512], BF16)
    biasr = [at(f"bias{l}s", [1, G], BF16) for l in range(L)]
    fc3br = at("fc3bs", [1, 512], BF16)
    ones64 = at("ones64s", [1, 64], BF16)
    ones128 = at("ones128s", [1, 128], BF16)
    id64 = at("id64s", [64, 64], F32)
    fc1b = at("fc1bs", [128, 4], F32)
    fc2b = at("fc2bs", [128, 4], F32)

    lstm_base = off[0]
    ring = [at(f"ring{r}", [128, KT, RING, BC], BF16) for r in range(2)]
    sig = at("sig", [64, 1536], F32)
    tang = at("tang", [64, 512], F32)
    tanc = at("tanc", [64, 512], F32)
    hnew = at("hnew", [64, 512], F32)
    tmp = at("tmp", [64, 512], F32)
    c_sb = [at(f"c{l}", [64, 512], F32) for l in range(L)]
    hsumT = at("hsumT", [128, KT, BC], BF16)
    h0T = at("h0T", [128, KT, BC], BF16)
    h1T = at("h1T", [128, KT, BC], BF16)
    # x_sb and xT are dead after step 0 starts: alias onto ring0 / hsumT
    rng0_off = ring[0].manual_sbuf_range[0]
    x_sb = at("x_sb", [64, 512], F32, offset=rng0_off)
    xT = at("xT", [128, KT, BC], BF16, offset=hsumT.manual_sbuf_range[0])
    assert off[0] <= nc.SBUF_PARTITION_SIZE_BYTES, off[0]

    # MLP working set aliases the LSTM working region (used strictly after it)
    off[0] = lstm_base
    actT = [at(f"actT{r}", [128, KT, 512], BF16) for r in range(2)]
    out1 = at("out1", [128, KT, 512], BF16)
    out2 = at("out2", [128, KT, 512], BF16)
    out3 = [at(f"out3_{m}", [128, 512], F32) for m in range(4)]
    assert off[0] <= nc.SBUF_PARTITION_SIZE_BYTES, off[0]

    with (
        nc.psum_tensor("P", [128, 4096], F32) as P,
        nc.semaphore("dma_in") as dma_in,
        nc.semaphore("dma_out") as dma_out,
        nc.semaphore("pe_s") as pe_s,
        nc.semaphore("act_s") as act_s,
        nc.semaphore("dve_s") as dve_s,
        nc.Block() as block,
    ):
        Pap = P.ap()
        G0 = Pap[0:64, 0:2048]
        G1 = Pap[0:64, 2048:4096]
        Tp = [Pap[0:128, 0:256], Pap[0:128, 2048:2304]]

        def stat_x(l, t):
            """stationary tiles ([128, BC] k-tiles) for the layer-l input."""
            if l == 1:
                return h0T
            return xT if t == 0 else hsumT

        # ---------------- SYNC: all DMA ----------------
        @block.sync
        def _(sync):
          for rep in range(reps):
            oD, oO, oP = rep * DVE_TOT, rep * DMA_OUT_TOT, rep * PE_TOT
            if rep > 0:
                sync.wait_ge(dma_out, rep * DMA_OUT_TOT)

            def load(dst_ap, src_ap):
                sync.dma_start(out=dst_ap, in_=src_ap).then_inc(dma_in, 16)

            load(x_sb.ap(), x_d.ap())
            for l in range(L):
                load(wih[l].ap(), wih_d[l].ap().rearrange("k p c -> p k c"))
                load(whh[l].ap(), whh_d[l].ap().rearrange("k p c -> p k c"))
                load(biasr[l].ap(), bias_d[l].ap())
            load(fc1w.ap(), fc1w_d.ap().rearrange("k p c -> p k c"))
            load(fc2w.ap(), fc2w_d.ap().rearrange("k p c -> p k c"))
            load(fc3w.ap(), fc3w_d.ap().rearrange("k p c -> p k c"))
            load(fc3br.ap(), fc3b_d.ap())
            load(ones64.ap(), ones64_d.ap())
            load(ones128.ap(), ones128_d.ap())
            load(id64.ap(), id64_d.ap())
            load(fc1b.ap(), fc1b_d.ap())
            load(fc2b.ap(), fc2b_d.ap())

            # ring flushes
            for f in range(NFLUSH):
                sync.wait_ge(dve_s, oD + dve_v(RING * f + RING - 1, "hsum"))
                sync.dma_start(
                    out=outsT_d.ap()[:, :, RING * f:RING * (f + 1), :]
                        .rearrange("k p t b -> p k t b"),
                    in_=ring[f % 2].ap(),
                ).then_inc(dma_out, 16)

            # MLP: prefetch loads + outputs
            def mload(j):
                sync.wait_ge(dma_out, oO + 16 * NFLUSH)
                if j >= 2:
                    sync.wait_ge(pe_s, oP + pe_mlp(j - 2, "fc1"))
                sync.dma_start(
                    out=actT[j % 2].ap(),
                    in_=outsT_d.ap()[:, :, RING * j:RING * (j + 1), :]
                        .rearrange("k p t b -> p k (t b)"),
                ).then_inc(dma_in, 16)

            mload(0)
            mload(1)
            for j in range(NCHUNK):
                for m in range(4):
                    sync.wait_ge(dve_s, oD + dve_mlp(j, m))
                    tt = 8 * j + 2 * m
                    sync.dma_start(
                        out=out_d.ap()[:, tt:tt + 2, :].rearrange("b u h -> u b h"),
                        in_=out3[m].ap(),
                    ).then_inc(dma_out, 16)
                if j + 2 < NCHUNK:
                    mload(j + 2)

        # ---------------- PE ----------------
        @block.tensor
        def _(tensor):
          for rep in range(reps):
            oI, oD, oA = rep * DMA_IN_TOT, rep * DVE_TOT, rep * ACT_TOT

            def bias_h_group(l, t, gp):
                """bias + recurrent-part matmuls for all 4 chunks of layer l."""
                hstat = h0T if l == 0 else h1T
                for c in range(NCH):
                    cs = slice(512 * c, 512 * (c + 1))
                    tensor.matmul(gp[:, cs], ones64.ap(), biasr[l].ap()[:, cs],
                                  start=True, stop=False)
                    if t > 0:
                        for k in range(KT):
                            tensor.matmul(gp[:, cs], hstat.ap()[:, k, :],
                                          whh[l].ap()[:, k, cs],
                                          start=False, stop=False)

            def x_group(l, t, gp):
                xstat = stat_x(l, t)
                for c in range(NCH):
                    cs = slice(512 * c, 512 * (c + 1))
                    for k in range(KT):
                        mm = tensor.matmul(gp[:, cs], xstat.ap()[:, k, :],
                                           wih[l].ap()[:, k, cs],
                                           start=False, stop=(k == KT - 1))
                    if c == 2:
                        mm.then_inc(pe_s, 1)
                mm.then_inc(pe_s, 1)

            def transpose_h(l):
                for c in range(4):
                    mm = tensor.transpose(Tp[l][:, 64 * c:64 * (c + 1)],
                                          hnew.ap()[:, 128 * c:128 * (c + 1)],
                                          id64.ap())
                mm.then_inc(pe_s, 1)

            # prologue: transpose x
            tensor.wait_ge(dma_in, oI + DMA_IN_PRE)
            for c in range(4):
                mm = tensor.transpose(Tp[0][:, 64 * c:64 * (c + 1)],
                                      x_sb.ap()[:, 128 * c:128 * (c + 1)],
                                      id64.ap())
            mm.then_inc(pe_s, 1)

            for t in range(T):
                # L0 x-part (stationary: xT at t=0 else hsumT(t-1))
                if t == 0:
                    # bias group must come after the xT copy frees Tp[0]
                    tensor.wait_ge(dve_s, oD + 1)
                    bias_h_group(0, 0, G0)
                else:
                    tensor.wait_ge(dve_s, oD + dve_v(t - 1, "hsum"))
                x_group(0, t, G0)
                # L1 bias + recurrent part
                if t > 0:
                    tensor.wait_ge(act_s, oA + act_v(t - 1, "tang1"))
                bias_h_group(1, t, G1)
                # transpose h0
                tensor.wait_ge(dve_s, oD + dve_v(t, "h0"))
                transpose_h(0)
                # L1 x-part (stationary: h0T(t))
                tensor.wait_ge(dve_s, oD + dve_v(t, "hT0"))
                x_group(1, t, G1)
                # transpose h1
                tensor.wait_ge(dve_s, oD + dve_v(t, "h1"))
                transpose_h(1)
                # next step L0 bias + h-part
                if t + 1 < T:
                    tensor.wait_ge(act_s, oA + act_v(t, "tang0"))
                    bias_h_group(0, t + 1, G0)

            # ---- MLP ----
            ps1 = [Pap[:, 512 * m:512 * (m + 1)] for m in range(4)]
            ps2 = [Pap[:, 2048 + 512 * m:2048 + 512 * (m + 1)] for m in range(4)]
            for j in range(NCHUNK):
                tensor.wait_ge(dma_in, oI + dma_in_load(j))
                if j >= 1:
                    tensor.wait_ge(dve_s, oD + dve_mlp(j - 1, 3))
                a = actT[j % 2]
                for m in range(4):
                    for k in range(KT):
                        mm = tensor.matmul(ps1[m], fc1w.ap()[:, k, 128 * m:128 * (m + 1)],
                                           a.ap()[:, k, :],
                                           start=(k == 0), stop=(k == KT - 1))
                mm.then_inc(pe_s, 1)
                tensor.wait_ge(act_s, oA + act_mlp(j, 1))
                for m in range(4):
                    for k in range(KT):
                        mm = tensor.matmul(ps2[m], fc2w.ap()[:, k, 128 * m:128 * (m + 1)],
                                           out1.ap()[:, k, :],
                                           start=(k == 0), stop=(k == KT - 1))
                mm.then_inc(pe_s, 1)
                tensor.wait_ge(act_s, oA + act_mlp(j, 2))
                for m in range(4):
                    tensor.matmul(ps1[m], ones128.ap(), fc3br.ap(),
                                  start=True, stop=False)
                    for k in range(KT):
                        mm = tensor.matmul(ps1[m],
                                           out2.ap()[:, k, 128 * m:128 * (m + 1)],
                                           fc3w.ap()[:, k, :],
                                           start=False, stop=(k == KT - 1))
                    mm.then_inc(pe_s, 1)

        # ---------------- ACT (scalar) ----------------
        @block.scalar
        def _(scalar):
          for rep in range(reps):
            oP, oD = rep * PE_TOT, rep * DVE_TOT
            for t in range(T):
                for l in range(L):
                    gp = G0 if l == 0 else G1
                    scalar.wait_ge(pe_s, oP + pe_v(t, f"l{l}c012"))
                    scalar.activation(sig.ap(), gp[:, 0:1536], AF.Sigmoid
                                      ).then_inc(act_s, 1)
                    scalar.wait_ge(pe_s, oP + pe_v(t, f"l{l}c3"))
                    scalar.activation(tang.ap(), gp[:, 1536:2048], AF.Tanh
                                      ).then_inc(act_s, 1)
                    scalar.wait_ge(dve_s, oD + dve_v(t, f"c{l}"))
                    scalar.activation(tanc.ap(), c_sb[l].ap(), AF.Tanh
                                      ).then_inc(act_s, 1)
            # MLP relu with fused per-partition bias
            for j in range(NCHUNK):
                scalar.wait_ge(pe_s, oP + pe_mlp(j, "fc1"))
                for m in range(4):
                    a = scalar.activation(out1.ap()[:, m, :],
                                          Pap[:, 512 * m:512 * (m + 1)], AF.Relu,
                                          bias=fc1b.ap()[:, m:m + 1])
                a.then_inc(act_s, 1)
                scalar.wait_ge(pe_s, oP + pe_mlp(j, "fc2"))
                for m in range(4):
                    a = scalar.activation(out2.ap()[:, m, :],
                                          Pap[:, 2048 + 512 * m:2048 + 512 * (m + 1)],
                                          AF.Relu, bias=fc2b.ap()[:, m:m + 1])
                a.then_inc(act_s, 1)

        # ---------------- DVE (vector) ----------------
        @block.vector
        def _(vector):
          for rep in range(reps):
            oP, oA, oO = rep * PE_TOT, rep * ACT_TOT, rep * DMA_OUT_TOT
            vector.wait_ge(pe_s, oP + 1)
            vector.tensor_copy(xT.ap().rearrange("p k b -> p (k b)"), Tp[0]
                               ).then_inc(dve_s, 1)
            for t in range(T):
                for l in range(L):
                    vector.wait_ge(act_s, oA + act_v(t, f"tang{l}"))
                    # tmp = i * tanh(g)
                    vector.tensor_tensor(tmp.ap(), sig.ap()[:, 0:512], tang.ap(), MUL)
                    if t == 0:
                        vector.tensor_copy(c_sb[l].ap(), tmp.ap()).then_inc(dve_s, 1)
                    else:
                        # c = c*f + tmp
                        vector.tensor_tensor(c_sb[l].ap(), c_sb[l].ap(),
                                             sig.ap()[:, 512:1024], MUL)
                        vector.tensor_tensor(c_sb[l].ap(), c_sb[l].ap(), tmp.ap(),
                                             ADD).then_inc(dve_s, 1)
                    vector.wait_ge(act_s, oA + act_v(t, f"tanc{l}"))
                    vector.tensor_tensor(hnew.ap(), sig.ap()[:, 1024:1536],
                                         tanc.ap(), MUL).then_inc(dve_s, 1)
                    vector.wait_ge(pe_s, oP + pe_v(t, f"l{l}T"))
                    hT = h0T if l == 0 else h1T
                    vector.tensor_copy(hT.ap().rearrange("p k b -> p (k b)"), Tp[l]
                                       ).then_inc(dve_s, 1)
                # hsum + ring write
                vector.tensor_tensor(hsumT.ap(), h0T.ap(), h1T.ap(), ADD)
                blk = t // RING
                if blk >= 2:
                    vector.wait_ge(dma_out, oO + 16 * (blk - 1))
                vector.tensor_copy(ring[blk % 2].ap()[:, :, t % RING, :],
                                   hsumT.ap()).then_inc(dve_s, 1)
            # MLP psum3 -> out3 copies
            for j in range(NCHUNK):
                for m in range(4):
                    vector.wait_ge(pe_s, oP + pe_mlp(j, m))
                    if j >= 1:
                        vector.wait_ge(dma_out, oO + dma_out_mlp(j - 1, m))
                    vector.tensor_copy(out3[m].ap(), Pap[:, 512 * m:512 * (m + 1)]
                                       ).then_inc(dve_s, 1)

    return nc


_PERM = None


def _gate_perm():
    # torch gate order (i, f, g, o) -> our column order (i, f, o, g)
    global _PERM
    if _PERM is None:
        i = np.arange(512)
        _PERM = np.concatenate([i, 512 + i, 1536 + i, 1024 + i])
    return _PERM


def _prep_inputs(x, W_ih, W_hh, b_ih, b_hh, fc1_w, fc1_b, fc2_w, fc2_b, fc3_w, fc3_b):
    perm = _gate_perm()
    bf = ml_dtypes.bfloat16
    common = {}
    for l in range(L):
        wt = np.ascontiguousarray(W_ih[l][perm].T)          # [512, 2048]
        common[f"wih{l}"] = wt.reshape(KT, 128, G).astype(bf)
        wt = np.ascontiguousarray(W_hh[l][perm].T)
        common[f"whh{l}"] = wt.reshape(KT, 128, G).astype(bf)
        common[f"bias{l}"] = (b_ih[l] + b_hh[l])[perm].reshape(1, G).astype(bf)
    common["fc1w"] = np.ascontiguousarray(fc1_w.T).reshape(KT, 128, 512).astype(bf)
    common["fc2w"] = np.ascontiguousarray(fc2_w.T).reshape(KT, 128, 512).astype(bf)
    common["fc3w"] = np.ascontiguousarray(fc3_w.T).reshape(KT, 128, 512).astype(bf)
    common["fc1b"] = np.ascontiguousarray(fc1_b.reshape(4, 128).T)
    common["fc2b"] = np.ascontiguousarray(fc2_b.reshape(4, 128).T)
    common["fc3b"] = fc3_b.reshape(1, 512).astype(bf)
    common["ones64"] = np.ones((1, 64), bf)
    common["ones128"] = np.ones((1, 128), bf)
    common["id64"] = np.eye(64, dtype=np.float32)
    in_maps = []
    for c in range(NCORES):
        m = dict(common)
        m["x"] = np.ascontiguousarray(x[BC * c:BC * (c + 1)])
        in_maps.append(m)
    return in_maps


_NC_CACHE = None


def kernel(**inputs):
    global _NC_CACHE
    if _NC_CACHE is None:
        _NC_CACHE = build_nc()
    nc = _NC_CACHE
    in_maps = _prep_inputs(**{k: np.asarray(v) for k, v in inputs.items()})
    res = run_bass_kernel_spmd(nc, in_maps, core_ids=list(range(NCORES)))
    out = np.concatenate([res.results[c]["out"] for c in range(NCORES)], axis=0)
    return out.astype(np.float32)



# revision 7
# speedup vs baseline: 1.8732x; 1.7382x over previous
"""DecoderLSTM Trainium2 kernel — 8-core data-parallel over batch.

Problem: 2-layer LSTM (H=512, B=512, T=128) where the step input is the sum of
the two layers' hidden states, followed by a 3-layer MLP head applied to the
[B, T, H] hidden-sum sequence.

Strategy (per core, B_c = 64 batch rows, zero collectives):
  - LSTM gates computed as g[B_c, 4H] with the *activations* stationary on the
    PE array ([K=128, M=64] tiles of x^T / h^T in bf16) and the *weights*
    streaming as the bf16 moving operand (2.4 GHz streaming) in 512-col chunks.
  - PE program keeps the engine hot: the next step's L0 bias+recurrent matmuls
    are issued *before* the h1 transpose so the engine never idles long enough
    to drop out of its high clock state.
  - Sigmoid is split (i,f | o) so the DVE c-chain starts ~1.7us earlier.
  - h_new is transposed back to [H, B_c] each step with bf16 PE transpose-mode
    matmuls; activations and hidden state are bf16, the cell state c stays f32.
  - Hidden sums are staged transposed in SBUF rings and flushed to a DRAM
    scratch every 8 steps; the MLP head then runs chunk-wise (512 rows at a
    time): fc1/fc2 weights-stationary with fused bias+ReLU on the scalar
    engine, fc3 activations-stationary so the result lands in [rows, H] layout
    for direct output DMA.
  - Raw bass (no Tile): explicit per-engine programs and semaphores.
"""

import ml_dtypes
import numpy as np

import concourse.bass as bass
import concourse.mybir as mybir
from concourse.bass_utils import run_bass_kernel_spmd

F32 = mybir.dt.float32
F32R = mybir.dt.float32r
BF16 = mybir.dt.bfloat16
AF = mybir.ActivationFunctionType
MUL = mybir.AluOpType.mult
ADD = mybir.AluOpType.add

NCORES = 8
B, H, T, L = 512, 512, 128, 2
BC = B // NCORES          # 64 batch rows per core
G = 4 * H                 # 2048 gate rows
KT = H // 128             # 4 K-tiles
NCH = 4                   # gate column chunks of 512
RING = 8                  # steps per outs ring flush
NFLUSH = T // RING        # 16
NCHUNK = (BC * T) // 512  # 16 MLP row chunks of 512

# ---- semaphore value schedules (pure functions of step/chunk) ----------------
# dve events per LSTM step: c0, h0, hT0, c1, h1, hT1, hsum  (1-based)
_DVE_E = {"c0": 1, "h0": 2, "hT0": 3, "c1": 4, "h1": 5, "hT1": 6, "hsum": 7}
# act events per LSTM step: per layer sig_if, tang, sig_o, tanc
_ACT_E = {"sigif0": 1, "tang0": 2, "sigo0": 3, "tanc0": 4,
          "sigif1": 5, "tang1": 6, "sigo1": 7, "tanc1": 8}
# pe events per LSTM step: per layer c01, c2, c3 (x_group), T (transpose)
_PE_E = {"l0c01": 1, "l0c2": 2, "l0c3": 3, "l0T": 4,
         "l1c01": 5, "l1c2": 6, "l1c3": 7, "l1T": 8}


def dve_v(t, e):
    return 1 + 7 * t + _DVE_E[e]          # +1 for initial xT copy


def act_v(t, e):
    return 8 * t + _ACT_E[e]


def pe_v(t, e):
    return 1 + 8 * t + _PE_E[e]           # +1 for x transposes


DVE_LSTM_END = 1 + 7 * T
ACT_LSTM_END = 8 * T
PE_LSTM_END = 1 + 8 * T


def dve_mlp(j, m):                        # after o3 copy (j, rowm m)
    return DVE_LSTM_END + 4 * j + m + 1


def act_mlp(j, which):                    # which: 1 = relu1, 2 = relu2
    return ACT_LSTM_END + 2 * j + which


# pe events per MLP chunk: fc1, fc2, fc3m0..fc3m3
def pe_mlp(j, e):
    base = PE_LSTM_END + 6 * j
    if e == "fc1":
        return base + 1
    if e == "fc2":
        return base + 2
    return base + 3 + int(e)


NPRE = 17                                 # preload DMA count
DMA_IN_PRE = 16 * NPRE


def dma_in_load(j):                       # after MLP chunk-j actT load
    return DMA_IN_PRE + 16 * (j + 1)


def dma_out_flush(f):
    return 16 * (f + 1)


def dma_out_mlp(j, m):
    return 16 * (NFLUSH + 4 * j + m + 1)


def build_nc(reps=1):
    assert reps == 1
    nc = bass.Bass("TRN2", target_bir_lowering=False, debug=False,
                   num_devices=NCORES)

    # ---- DRAM I/O ----
    x_d = nc.dram_tensor("x", [BC, H], F32, kind="ExternalInput")
    wih_d = [nc.dram_tensor(f"wih{l}", [KT, 128, G], BF16, kind="ExternalInput") for l in range(L)]
    whh_d = [nc.dram_tensor(f"whh{l}", [KT, 128, G], BF16, kind="ExternalInput") for l in range(L)]
    bias_d = [nc.dram_tensor(f"bias{l}", [1, G], BF16, kind="ExternalInput") for l in range(L)]
    fc1w_d = nc.dram_tensor("fc1w", [KT, 128, 512], BF16, kind="ExternalInput")
    fc2w_d = nc.dram_tensor("fc2w", [KT, 128, 512], BF16, kind="ExternalInput")
    fc3w_d = nc.dram_tensor("fc3w", [KT, 128, 512], BF16, kind="ExternalInput")
    fc1b_d = nc.dram_tensor("fc1b", [128, 4], F32, kind="ExternalInput")
    fc2b_d = nc.dram_tensor("fc2b", [128, 4], F32, kind="ExternalInput")
    fc3b_d = nc.dram_tensor("fc3b", [1, 512], BF16, kind="ExternalInput")
    ones64_d = nc.dram_tensor("ones64", [1, 64], BF16, kind="ExternalInput")
    ones128_d = nc.dram_tensor("ones128", [1, 128], BF16, kind="ExternalInput")
    id64_d = nc.dram_tensor("id64", [64, 64], F32, kind="ExternalInput")
    id64b_d = nc.dram_tensor("id64b", [64, 64], BF16, kind="ExternalInput")
    outsT_d = nc.dram_tensor("outsT", [KT, 128, T, BC], BF16, kind="Internal")
    out_d = nc.dram_tensor("out", [BC, T, H], F32, kind="ExternalOutput")

    # ---- hand-drawn SBUF map (per-partition byte offsets) ----
    off = [(nc.sbuf_base + 63) // 64 * 64]

    def at(name, shape, dtype, align=32, offset=None):
        o = (off[0] + align - 1) // align * align if offset is None else offset
        h = nc.alloc_sbuf_tensor_at(name, shape, dtype, offset=o)
        sz = int(np.prod(shape[1:])) * mybir.dt.size(dtype)
        if offset is None:
            off[0] = o + sz
        return h

    wih = [at(f"wih{l}s", [128, KT, G], BF16) for l in range(L)]
    whh = [at(f"whh{l}s", [128, KT, G], BF16) for l in range(L)]
    fc1w = at("fc1ws", [128, KT, 512], BF16)
    fc2w = at("fc2ws", [128, KT, 512], BF16)
    fc3w = at("fc3ws", [128, KT, 512], BF16)
    biasr = [at(f"bias{l}s", [1, G], BF16) for l in range(L)]
    fc3br = at("fc3bs", [1, 512], BF16)
    ones64 = at("ones64s", [1, 64], BF16)
    ones128 = at("ones128s", [1, 128], BF16)
    id64 = at("id64s", [64, 64], F32)
    id64b = at("id64bs", [64, 64], BF16)
    fc1b = at("fc1bs", [128, 4], F32)
    fc2b = at("fc2bs", [128, 4], F32)

    lstm_base = off[0]
    ring = [at(f"ring{r}", [128, KT, RING, BC], BF16) for r in range(2)]
    sig = at("sig", [64, 1536], BF16)
    tang = at("tang", [64, 512], BF16)
    tanc = at("tanc", [64, 512], BF16)
    hnew = at("hnew", [64, 512], BF16)
    tmp = at("tmp", [64, 512], BF16)
    c_sb = [at(f"c{l}", [64, 512], F32) for l in range(L)]
    hsumT = at("hsumT", [128, KT, BC], BF16)
    h0T = at("h0T", [128, KT, BC], BF16)
    h1T = at("h1T", [128, KT, BC], BF16)
    # x_sb and xT are dead after step 0 starts: alias onto ring0 / hsumT
    rng0_off = ring[0].manual_sbuf_range[0]
    x_sb = at("x_sb", [64, 512], F32, offset=rng0_off)
    xT = at("xT", [128, KT, BC], BF16, offset=hsumT.manual_sbuf_range[0])
    assert off[0] <= nc.SBUF_PARTITION_SIZE_BYTES, off[0]

    # MLP working set aliases the LSTM working region (used strictly after it)
    off[0] = lstm_base
    actT = [at(f"actT{r}", [128, KT, 512], BF16) for r in range(2)]
    out1 = at("out1", [128, KT, 512], BF16)
    out2 = at("out2", [128, KT, 512], BF16)
    out3 = [at(f"out3_{m}", [128, 512], F32) for m in range(4)]
    assert off[0] <= nc.SBUF_PARTITION_SIZE_BYTES, off[0]

    with (
        nc.psum_tensor("P", [128, 4096], F32) as P,
        nc.semaphore("dma_in") as dma_in,
        nc.semaphore("dma_out") as dma_out,
        nc.semaphore("pe_s") as pe_s,
        nc.semaphore("act_s") as act_s,
        nc.semaphore("dve_s") as dve_s,
        nc.Block() as block,
    ):
        Pap = P.ap()
        G0 = Pap[0:64, 0:2048]
        G1 = Pap[0:64, 2048:4096]
        Tp32 = Pap[0:128, 0:256]                        # x prologue (f32)
        # bf16 transpose staging: aliases the i-chunk head of G0 / G1
        TpB = [Pap[0:128, 0:128].bitcast(BF16),
               Pap[0:128, 2048:2176].bitcast(BF16)]     # each [128, 256] bf16

        def stat_x(l, t):
            """stationary tiles ([128, BC] k-tiles) for the layer-l input."""
            if l == 1:
                return h0T
            return xT if t == 0 else hsumT

        # ---------------- SYNC: all DMA ----------------
        @block.sync
        def _(sync):
            def load(dst_ap, src_ap):
                sync.dma_start(out=dst_ap, in_=src_ap).then_inc(dma_in, 16)

            load(x_sb.ap(), x_d.ap())
            for l in range(L):
                load(wih[l].ap(), wih_d[l].ap().rearrange("k p c -> p k c"))
                load(whh[l].ap(), whh_d[l].ap().rearrange("k p c -> p k c"))
                load(biasr[l].ap(), bias_d[l].ap())
            load(fc1w.ap(), fc1w_d.ap().rearrange("k p c -> p k c"))
            load(fc2w.ap(), fc2w_d.ap().rearrange("k p c -> p k c"))
            load(fc3w.ap(), fc3w_d.ap().rearrange("k p c -> p k c"))
            load(fc3br.ap(), fc3b_d.ap())
            load(ones64.ap(), ones64_d.ap())
            load(ones128.ap(), ones128_d.ap())
            load(id64.ap(), id64_d.ap())
            load(id64b.ap(), id64b_d.ap())
            load(fc1b.ap(), fc1b_d.ap())
            load(fc2b.ap(), fc2b_d.ap())

            # ring flushes
            for f in range(NFLUSH):
                sync.wait_ge(dve_s, dve_v(RING * f + RING - 1, "hsum"))
                sync.dma_start(
                    out=outsT_d.ap()[:, :, RING * f:RING * (f + 1), :]
                        .rearrange("k p t b -> p k t b"),
                    in_=ring[f % 2].ap(),
                ).then_inc(dma_out, 16)

            # MLP: prefetch loads + outputs
            def mload(j):
                sync.wait_ge(dma_out, 16 * NFLUSH)
                if j >= 2:
                    sync.wait_ge(pe_s, pe_mlp(j - 2, "fc1"))
                sync.dma_start(
                    out=actT[j % 2].ap(),
                    in_=outsT_d.ap()[:, :, RING * j:RING * (j + 1), :]
                        .rearrange("k p t b -> p k (t b)"),
                ).then_inc(dma_in, 16)

            mload(0)
            mload(1)
            for j in range(NCHUNK):
                for m in range(4):
                    sync.wait_ge(dve_s, dve_mlp(j, m))
                    tt = 8 * j + 2 * m
                    sync.dma_start(
                        out=out_d.ap()[:, tt:tt + 2, :].rearrange("b u h -> u b h"),
                        in_=out3[m].ap(),
                    ).then_inc(dma_out, 16)
                if j + 2 < NCHUNK:
                    mload(j + 2)

        # ---------------- PE ----------------
        @block.tensor
        def _(tensor):
            def bias_group(l, gp):
                for c in range(NCH):
                    cs = slice(512 * c, 512 * (c + 1))
                    tensor.matmul(gp[:, cs], ones64.ap(), biasr[l].ap()[:, cs],
                                  start=True, stop=False)

            def h_group(l, t, gp, chunks):
                hstat = h0T if l == 0 else h1T
                for c in chunks:
                    cs = slice(512 * c, 512 * (c + 1))
                    for k in range(KT):
                        tensor.matmul(gp[:, cs], hstat.ap()[:, k, :],
                                      whh[l].ap()[:, k, cs],
                                      start=False, stop=False)

            def x_group(l, t, gp):
                xstat = stat_x(l, t)
                for c in range(NCH):
                    cs = slice(512 * c, 512 * (c + 1))
                    for k in range(KT):
                        mm = tensor.matmul(gp[:, cs], xstat.ap()[:, k, :],
                                           wih[l].ap()[:, k, cs],
                                           start=False, stop=(k == KT - 1))
                    if c >= 1:
                        mm.then_inc(pe_s, 1)    # c01 / c2 / c3

            def transpose_bf(l):
                for c in range(4):
                    mm = tensor.transpose(TpB[l][:, 64 * c:64 * (c + 1)],
                                          hnew.ap()[:, 128 * c:128 * (c + 1)],
                                          id64b.ap())
                mm.then_inc(pe_s, 1)

            # prologue: transpose x (f32)
            tensor.wait_ge(dma_in, DMA_IN_PRE)
            for c in range(4):
                mm = tensor.transpose(Tp32[:, 64 * c:64 * (c + 1)],
                                      x_sb.ap()[:, 128 * c:128 * (c + 1)],
                                      id64.ap())
            mm.then_inc(pe_s, 1)

            for t in range(T):
                # [A] L0 x-part (stationary: xT at t=0 else hsumT(t-1))
                if t == 0:
                    tensor.wait_ge(dve_s, 1)    # xT copy frees Tp32
                    bias_group(0, G0)
                else:
                    tensor.wait_ge(dve_s, dve_v(t - 1, "hsum"))
                x_group(0, t, G0)
                # [B] L1 bias + recurrent part
                if t == 0:
                    bias_group(1, G1)
                else:
                    tensor.wait_ge(act_s, act_v(t - 1, "sigo1"))
                    bias_group(1, G1)
                    h_group(1, t, G1, range(NCH))
                # [C] transpose h0
                tensor.wait_ge(dve_s, dve_v(t, "h0"))
                transpose_bf(0)
                # [D] L1 x-part (stationary: h0T(t))
                tensor.wait_ge(dve_s, dve_v(t, "hT0"))
                x_group(1, t, G1)
                # [F1] next step L0 bias + h-part chunks 0,1 (keeps PE hot
                # while the L1 activation chain runs)
                if t + 1 < T:
                    tensor.wait_ge(act_s, act_v(t, "sigo0"))
                    bias_group(0, G0)
                    h_group(0, t + 1, G0, (0, 1))
                # [E] transpose h1
                tensor.wait_ge(dve_s, dve_v(t, "h1"))
                transpose_bf(1)
                # [F2] next step L0 h-part chunks 2,3
                if t + 1 < T:
                    h_group(0, t + 1, G0, (2, 3))

            # ---- MLP ----
            ps1 = [Pap[:, 512 * m:512 * (m + 1)] for m in range(4)]
            ps2 = [Pap[:, 2048 + 512 * m:2048 + 512 * (m + 1)] for m in range(4)]
            for j in range(NCHUNK):
                tensor.wait_ge(dma_in, dma_in_load(j))
                if j >= 1:
                    tensor.wait_ge(dve_s, dve_mlp(j - 1, 3))
                a = actT[j % 2]
                for m in range(4):
                    for k in range(KT):
                        mm = tensor.matmul(ps1[m], fc1w.ap()[:, k, 128 * m:128 * (m + 1)],
                                           a.ap()[:, k, :],
                                           start=(k == 0), stop=(k == KT - 1))
                mm.then_inc(pe_s, 1)
                tensor.wait_ge(act_s, act_mlp(j, 1))
                for m in range(4):
                    for k in range(KT):
                        mm = tensor.matmul(ps2[m], fc2w.ap()[:, k, 128 * m:128 * (m + 1)],
                                           out1.ap()[:, k, :],
                                           start=(k == 0), stop=(k == KT - 1))
                mm.then_inc(pe_s, 1)
                tensor.wait_ge(act_s, act_mlp(j, 2))
                for m in range(4):
                    tensor.matmul(ps1[m], ones128.ap(), fc3br.ap(),
                                  start=True, stop=False)
                    for k in range(KT):
                        mm = tensor.matmul(ps1[m],
                                           out2.ap()[:, k, 128 * m:128 * (m + 1)],
                                           fc3w.ap()[:, k, :],
                                           start=False, stop=(k == KT - 1))
                    mm.then_inc(pe_s, 1)

        # ---------------- ACT (scalar) ----------------
        @block.scalar
        def _(scalar):
            for t in range(T):
                for l in range(L):
                    gp = G0 if l == 0 else G1
                    scalar.wait_ge(pe_s, pe_v(t, f"l{l}c01"))
                    scalar.activation(sig.ap()[:, 0:1024], gp[:, 0:1024],
                                      AF.Sigmoid).then_inc(act_s, 1)
                    scalar.wait_ge(pe_s, pe_v(t, f"l{l}c3"))
                    scalar.activation(tang.ap(), gp[:, 1536:2048], AF.Tanh
                                      ).then_inc(act_s, 1)
                    scalar.activation(sig.ap()[:, 1024:1536], gp[:, 1024:1536],
                                      AF.Sigmoid).then_inc(act_s, 1)
                    scalar.wait_ge(dve_s, dve_v(t, f"c{l}"))
                    scalar.activation(tanc.ap(), c_sb[l].ap(), AF.Tanh
                                      ).then_inc(act_s, 1)
            # MLP relu with fused per-partition bias
            for j in range(NCHUNK):
                scalar.wait_ge(pe_s, pe_mlp(j, "fc1"))
                for m in range(4):
                    a = scalar.activation(out1.ap()[:, m, :],
                                          Pap[:, 512 * m:512 * (m + 1)], AF.Relu,
                                          bias=fc1b.ap()[:, m:m + 1])
                a.then_inc(act_s, 1)
                scalar.wait_ge(pe_s, pe_mlp(j, "fc2"))
                for m in range(4):
                    a = scalar.activation(out2.ap()[:, m, :],
                                          Pap[:, 2048 + 512 * m:2048 + 512 * (m + 1)],
                                          AF.Relu, bias=fc2b.ap()[:, m:m + 1])
                a.then_inc(act_s, 1)

        # ---------------- DVE (vector) ----------------
        @block.vector
        def _(vector):
            vector.wait_ge(pe_s, 1)
            vector.tensor_copy(xT.ap().rearrange("p k b -> p (k b)"), Tp32
                               ).then_inc(dve_s, 1)
            for t in range(T):
                for l in range(L):
                    if t > 0:
                        # c *= sig(f)  (can start as soon as sig_if lands)
                        vector.wait_ge(act_s, act_v(t, f"sigif{l}"))
                        vector.tensor_tensor(c_sb[l].ap(), c_sb[l].ap(),
                                             sig.ap()[:, 512:1024], MUL)
                    vector.wait_ge(act_s, act_v(t, f"tang{l}"))
                    # tmp = sig(i) * tanh(g)
                    vector.tensor_tensor(tmp.ap(), sig.ap()[:, 0:512], tang.ap(),
                                         MUL)
                    if t == 0:
                        vector.tensor_copy(c_sb[l].ap(), tmp.ap()).then_inc(dve_s, 1)
                    else:
                        vector.tensor_tensor(c_sb[l].ap(), c_sb[l].ap(), tmp.ap(),
                                             ADD).then_inc(dve_s, 1)
                    vector.wait_ge(act_s, act_v(t, f"tanc{l}"))
                    vector.tensor_tensor(hnew.ap(), sig.ap()[:, 1024:1536],
                                         tanc.ap(), MUL).then_inc(dve_s, 1)
                    vector.wait_ge(pe_s, pe_v(t, f"l{l}T"))
                    hT = h0T if l == 0 else h1T
                    vector.tensor_copy(hT.ap().rearrange("p k b -> p (k b)"), TpB[l]
                                       ).then_inc(dve_s, 1)
                # hsum + ring write
                vector.tensor_tensor(hsumT.ap(), h0T.ap(), h1T.ap(), ADD)
                blk = t // RING
                if blk >= 2:
                    vector.wait_ge(dma_out, 16 * (blk - 1))
                vector.tensor_copy(ring[blk % 2].ap()[:, :, t % RING, :],
                                   hsumT.ap()).then_inc(dve_s, 1)
            # MLP psum3 -> out3 copies
            for j in range(NCHUNK):
                for m in range(4):
                    vector.wait_ge(pe_s, pe_mlp(j, m))
                    if j >= 1:
                        vector.wait_ge(dma_out, dma_out_mlp(j - 1, m))
                    vector.tensor_copy(out3[m].ap(), Pap[:, 512 * m:512 * (m + 1)]
                                       ).then_inc(dve_s, 1)

    return nc


_PERM = None


def _gate_perm():
    # torch gate order (i, f, g, o) -> our column order (i, f, o, g)
    global _PERM
    if _PERM is None:
        i = np.arange(512)
        _PERM = np.concatenate([i, 512 + i, 1536 + i, 1024 + i])
    return _PERM


def _prep_inputs(x, W_ih, W_hh, b_ih, b_hh, fc1_w, fc1_b, fc2_w, fc2_b, fc3_w, fc3_b):
    perm = _gate_perm()
    bf = ml_dtypes.bfloat16
    common = {}
    for l in range(L):
        wt = np.ascontiguousarray(W_ih[l][perm].T)          # [512, 2048]
        common[f"wih{l}"] = wt.reshape(KT, 128, G).astype(bf)
        wt = np.ascontiguousarray(W_hh[l][perm].T)
        common[f"whh{l}"] = wt.reshape(KT, 128, G).astype(bf)
        common[f"bias{l}"] = (b_ih[l] + b_hh[l])[perm].reshape(1, G).astype(bf)
    common["fc1w"] = np.ascontiguousarray(fc1_w.T).reshape(KT, 128, 512).astype(bf)
    common["fc2w"] = np.ascontiguousarray(fc2_w.T).reshape(KT, 128, 512).astype(bf)
    common["fc3w"] = np.ascontiguousarray(fc3_w.T).reshape(KT, 128, 512).astype(bf)
    common["fc1b"] = np.ascontiguousarray(fc1_b.reshape(4, 128).T)
    common["fc2b"] = np.ascontiguousarray(fc2_b.reshape(4, 128).T)
    common["fc3b"] = fc3_b.reshape(1, 512).astype(bf)
    common["ones64"] = np.ones((1, 64), bf)
    common["ones128"] = np.ones((1, 128), bf)
    common["id64"] = np.eye(64, dtype=np.float32)
    common["id64b"] = np.eye(64).astype(bf)
    in_maps = []
    for c in range(NCORES):
        m = dict(common)
        m["x"] = np.ascontiguousarray(x[BC * c:BC * (c + 1)])
        in_maps.append(m)
    return in_maps


_NC_CACHE = None


def kernel(**inputs):
    global _NC_CACHE
    if _NC_CACHE is None:
        _NC_CACHE = build_nc()
    nc = _NC_CACHE
    in_maps = _prep_inputs(**{k: np.asarray(v) for k, v in inputs.items()})
    res = run_bass_kernel_spmd(nc, in_maps, core_ids=list(range(NCORES)))
    out = np.concatenate([res.results[c]["out"] for c in range(NCORES)], axis=0)
    return out.astype(np.float32)


# revision 12
# speedup vs baseline: 2.0268x; 1.0820x over previous
"""DecoderLSTM Trainium2 kernel — 8-core data-parallel over batch.

Problem: 2-layer LSTM (H=512, B=512, T=128) where the step input is the sum of
the two layers' hidden states, followed by a 3-layer MLP head applied to the
[B, T, H] hidden-sum sequence.

Strategy (per core, B_c = 64 batch rows, zero collectives):
  - LSTM gates computed as g[B_c, 4H] with the *activations* stationary on the
    PE array ([K=128, M=64] bf16 tiles of x^T / h^T) and the *weights*
    streaming as the bf16 moving operand in 512-col chunks.
  - Layer 0 gates accumulate in PSUM partitions 0-63, layer 1 in partitions
    64-127 (col tile_position 64), so PSUM banks 4-7 stay free for the MLP.
  - All activation/state buffers are [128, *] with layer 0 in the lower and
    layer 1 in the upper partition half; cell state c stays f32.
  - The MLP head consumes the hidden-sum ring directly from SBUF (no DRAM
    round-trip) and its matmul groups are interleaved into the LSTM steps as
    PE filler, so the engine never idles long enough to downclock.
  - PE program order per step: x0 | bias1+h1 | transpose h0 | x1 |
    bias0+h0(next, half) | mlp | transpose h1 | h0(next, half) | mlp.
  - Raw bass (no Tile): explicit per-engine programs and semaphores, emitted
    from a symbolic two-pass schedule.
"""

import ml_dtypes
import numpy as np

import concourse.bass as bass
import concourse.mybir as mybir
from concourse.bass_utils import run_bass_kernel_spmd

F32 = mybir.dt.float32
BF16 = mybir.dt.bfloat16
AF = mybir.ActivationFunctionType
MUL = mybir.AluOpType.mult
ADD = mybir.AluOpType.add

NCORES = 8
B, H, T, L = 512, 512, 128, 2
BC = B // NCORES          # 64 batch rows per core
G = 4 * H                 # 2048 gate rows
KT = H // 128             # 4 K-tiles
NCH = 4                   # gate column chunks of 512
RING = 8                  # steps per ring buffer
NCHUNK = (BC * T) // 512  # 16 MLP row chunks of 512

N_LSTM_LOAD = 10
N_MLP_LOAD = 7


def build_nc(reps=1):
    assert reps == 1
    nc = bass.Bass("TRN2", target_bir_lowering=False, debug=False,
                   num_devices=NCORES)

    # ---- DRAM I/O ----
    x_d = nc.dram_tensor("x", [BC, H], F32, kind="ExternalInput")
    wih_d = [nc.dram_tensor(f"wih{l}", [KT, 128, G], BF16, kind="ExternalInput") for l in range(L)]
    whh_d = [nc.dram_tensor(f"whh{l}", [KT, 128, G], BF16, kind="ExternalInput") for l in range(L)]
    bias_d = [nc.dram_tensor(f"bias{l}", [1, G], BF16, kind="ExternalInput") for l in range(L)]
    fc1w_d = nc.dram_tensor("fc1w", [KT, 128, 512], BF16, kind="ExternalInput")
    fc2w_d = nc.dram_tensor("fc2w", [KT, 128, 512], BF16, kind="ExternalInput")
    fc3w_d = nc.dram_tensor("fc3w", [KT, 128, 512], BF16, kind="ExternalInput")
    fc1b_d = nc.dram_tensor("fc1b", [128, 4], F32, kind="ExternalInput")
    fc2b_d = nc.dram_tensor("fc2b", [128, 4], F32, kind="ExternalInput")
    fc3b_d = nc.dram_tensor("fc3b", [1, 512], BF16, kind="ExternalInput")
    ones64_d = nc.dram_tensor("ones64", [1, 64], BF16, kind="ExternalInput")
    ones128_d = nc.dram_tensor("ones128", [1, 128], BF16, kind="ExternalInput")
    id64_d = nc.dram_tensor("id64", [64, 64], F32, kind="ExternalInput")
    id64b_d = nc.dram_tensor("id64b", [128, 64], BF16, kind="ExternalInput")
    out_d = nc.dram_tensor("out", [BC, T, H], F32, kind="ExternalOutput")

    # ---- SBUF map ----
    off = [(nc.sbuf_base + 63) // 64 * 64]

    def at(name, shape, dtype, align=32):
        o = (off[0] + align - 1) // align * align
        h = nc.alloc_sbuf_tensor_at(name, shape, dtype, offset=o)
        off[0] = o + int(np.prod(shape[1:])) * mybir.dt.size(dtype)
        return h

    wih = [at(f"wih{l}s", [128, KT, G], BF16) for l in range(L)]
    whh = [at(f"whh{l}s", [128, KT, G], BF16) for l in range(L)]
    fc1w = at("fc1ws", [128, KT, 512], BF16)
    fc2w = at("fc2ws", [128, KT, 512], BF16)
    fc3w = at("fc3ws", [128, KT, 512], BF16)
    biasr = [at(f"bias{l}s", [1, G], BF16) for l in range(L)]
    fc3br = at("fc3bs", [1, 512], BF16)
    ones64 = at("ones64s", [1, 64], BF16)
    ones128 = at("ones128s", [1, 128], BF16)
    id64 = at("id64s", [64, 64], F32)
    id64b = at("id64bs", [128, 64], BF16)
    fc1b = at("fc1bs", [128, 4], F32)
    fc2b = at("fc2bs", [128, 4], F32)
    ring = [at(f"ring{r}", [128, KT, RING, BC], BF16) for r in range(2)]
    sig = at("sig", [128, 1536], BF16)
    tang = at("tang", [128, 512], BF16)
    tanc = at("tanc", [128, 512], BF16)
    hnew = at("hnew", [128, 512], BF16)
    tmp = at("tmp", [128, 512], BF16)
    c_sb = at("c_sb", [128, 512], F32)
    hsumT = at("hsumT", [128, KT, BC], BF16)
    h0T = at("h0T", [128, KT, BC], BF16)
    h1T = at("h1T", [128, KT, BC], BF16)
    xT = at("xT", [128, KT, BC], BF16)
    x_sb = at("x_sb", [64, 512], F32)
    out1 = at("out1", [128, KT, 512], BF16)
    out2 = at("out2", [128, KT, 512], BF16)
    out3 = [at(f"out3_{m}", [128, 512], F32) for m in range(4)]
    assert off[0] <= nc.SBUF_PARTITION_SIZE_BYTES, off[0]

    # ---- symbolic schedules (two-pass: build op lists, then emit) ----
    val = {"pe": {}, "act": {}, "dve": {}, "dout": {}}
    cnt = {"pe": 0, "act": 0, "dve": 0, "dout": 0}
    progs = {"pe": [], "act": [], "dve": [], "sync": []}

    def w(eng, sem, key):
        progs[eng].append(("w", sem, key))

    def op(eng, fn, sem=None, key=None, n=1):
        if sem is not None:
            cnt[sem] += n
            if key is not None:
                assert key not in val[sem], key
                val[sem][key] = cnt[sem]
        progs[eng].append(("o", fn, sem, n))

    # --- PSUM layout (built at emit time; descriptors here) ---
    # G0: [0:64, 0:2048]   G1: [64:128, 0:2048]
    # psAB: banks 4,5 ([:, 2048:2560], [:, 2560:3072])
    # Tp32: [:, 3072:3328] f32 (x prologue)
    # TpB[l]: [:, 3328+128*l : ...] bitcast bf16 [128, 256]

    lsl = [slice(0, 64), slice(64, 128)]    # layer partition slices

    # ================= PE program =================
    def pe_bias(l):
        def f(e, P):
            gp = P["G"][l]
            for c in range(NCH):
                cs = slice(512 * c, 512 * (c + 1))
                mm = e.matmul(gp[:, cs], ones64.ap(), biasr[l].ap()[:, cs],
                              start=True, stop=False)
            return mm
        return f

    def pe_h(l, chunks):
        hstat = h0T if l == 0 else h1T
        def f(e, P):
            gp = P["G"][l]
            for c in chunks:
                cs = slice(512 * c, 512 * (c + 1))
                for k in range(KT):
                    mm = e.matmul(gp[:, cs], hstat.ap()[:, k, :],
                                  whh[l].ap()[:, k, cs], start=False, stop=False)
            return mm
        return f

    def pe_x(l, t, chunks):
        xstat = h0T if l == 1 else (xT if t == 0 else hsumT)
        def f(e, P):
            gp = P["G"][l]
            for c in chunks:
                cs = slice(512 * c, 512 * (c + 1))
                for k in range(KT):
                    mm = e.matmul(gp[:, cs], xstat.ap()[:, k, :],
                                  wih[l].ap()[:, k, cs],
                                  start=False, stop=(k == KT - 1))
            return mm
        return f

    def pe_transpose(l):
        def f(e, P):
            for c in range(4):
                mm = e.transpose(P["TpB"][l][:, 64 * c:64 * (c + 1)],
                                 hnew.ap()[lsl[l], 128 * c:128 * (c + 1)],
                                 id64b.ap()[lsl[l], :])
            return mm
        return f

    def pe_xpro():
        def f(e, P):
            for c in range(4):
                mm = e.transpose(P["Tp32"][:, 64 * c:64 * (c + 1)],
                                 x_sb.ap()[:, 128 * c:128 * (c + 1)], id64.ap())
            return mm
        return f

    def pe_fc1(j, m):
        def f(e, P):
            ps = P["ps"][m % 2]
            for k in range(KT):
                mm = e.matmul(ps, fc1w.ap()[:, k, 128 * m:128 * (m + 1)],
                              ring[j % 2].ap()[:, k, :, :],
                              start=(k == 0), stop=(k == KT - 1))
            return mm
        return f

    def pe_fc2(j, m):
        def f(e, P):
            ps = P["ps"][m % 2]
            for k in range(KT):
                mm = e.matmul(ps, fc2w.ap()[:, k, 128 * m:128 * (m + 1)],
                              out1.ap()[:, k, :], start=(k == 0), stop=(k == KT - 1))
            return mm
        return f

    def pe_fc3(j, m):
        def f(e, P):
            ps = P["ps"][m % 2]
            e.matmul(ps, ones128.ap(), fc3br.ap(), start=True, stop=False)
            for k in range(KT):
                mm = e.matmul(ps, out2.ap()[:, k, 128 * m:128 * (m + 1)],
                              fc3w.ap()[:, k, :], start=False, stop=(k == KT - 1))
            return mm
        return f

    # MLP group table: 12 groups per chunk, placed at 2 slots per step in the
    # window steps 8j+8 .. 8j+13 (chunk 15 trails after the loop).
    def mlp_group(j, g):
        kind, m = ("fc1", "fc2", "fc3")[g // 4], g % 4
        if kind == "fc1":
            w("pe", "dve", f"hsum@{8 * j + 7}")
            if j == 0 and g == 0:
                w("pe", "mlp_in", 16 * N_MLP_LOAD)
            if m == 0 and j > 0:
                w("pe", "dve", f"o3m2@{j - 1}")
            if m == 1 and j > 0:
                w("pe", "dve", f"o3m3@{j - 1}")
            if m == 2:
                w("pe", "act", f"relu1m0@{j}")
            if m == 3:
                w("pe", "act", f"relu1m1@{j}")
            op("pe", pe_fc1(j, m), "pe", f"fc1m{m}@{j}")
        elif kind == "fc2":
            w("pe", "act", f"relu1m3@{j}")
            if m == 2:
                w("pe", "act", f"relu2m0@{j}")
            if m == 3:
                w("pe", "act", f"relu2m1@{j}")
            op("pe", pe_fc2(j, m), "pe", f"fc2m{m}@{j}")
        else:
            w("pe", "act", f"relu2m3@{j}")
            if m == 2:
                w("pe", "dve", f"o3m0@{j}")
            if m == 3:
                w("pe", "dve", f"o3m1@{j}")
            op("pe", pe_fc3(j, m), "pe", f"fc3m{m}@{j}")

    def mlp_slots(t):
        """(chunk, group) list for the two insertion points of step t."""
        j, s = (t - 8) // 8, (t - 8) % 8
        if t >= 8 and j < NCHUNK - 1 and s < 6:
            return [(j, 2 * s), (j, 2 * s + 1)]
        return []

    # prologue
    w("pe", "lstm_in", 16 * N_LSTM_LOAD)
    op("pe", pe_xpro(), "pe", "xTp")

    for t in range(T):
        slots = mlp_slots(t)
        # [A] L0 x-part
        if t == 0:
            w("pe", "dve", "xT")
            op("pe", pe_bias(0))
        else:
            w("pe", "dve", f"hsum@{t - 1}")
        op("pe", pe_x(0, t, (0,)))
        op("pe", pe_x(0, t, (1,)), "pe", f"c01_0@{t}")
        op("pe", pe_x(0, t, (2,)), "pe", f"c2_0@{t}")
        op("pe", pe_x(0, t, (3,)), "pe", f"c3_0@{t}")
        # [B] L1 bias + recurrent
        if t == 0:
            op("pe", pe_bias(1))
        else:
            w("pe", "act", f"sigo1@{t - 1}")
            op("pe", pe_bias(1))
            op("pe", pe_h(1, range(NCH)))
        # [C] transpose h0
        w("pe", "dve", f"h0@{t}")
        op("pe", pe_transpose(0), "pe", f"T0@{t}")
        # [D] L1 x-part
        w("pe", "dve", f"hT0@{t}")
        op("pe", pe_x(1, t, (0,)))
        op("pe", pe_x(1, t, (1,)), "pe", f"c01_1@{t}")
        op("pe", pe_x(1, t, (2,)), "pe", f"c2_1@{t}")
        op("pe", pe_x(1, t, (3,)), "pe", f"c3_1@{t}")
        # [F1] next-step L0 bias + h chunks 0,1
        if t + 1 < T:
            w("pe", "act", f"sigo0@{t}")
            op("pe", pe_bias(0))
            op("pe", pe_h(0, (0, 1)))
        if slots:
            mlp_group(*slots[0])
        # [E] transpose h1
        w("pe", "dve", f"h1@{t}")
        op("pe", pe_transpose(1), "pe", f"T1@{t}")
        # [F2] next-step L0 h chunks 2,3
        if t + 1 < T:
            op("pe", pe_h(0, (2, 3)))
        if slots:
            mlp_group(*slots[1])
    for g in range(12):
        mlp_group(NCHUNK - 1, g)

    # ================= ACT program =================
    def act_sig(l, lo, hi):
        def f(e, P):
            return e.activation(sig.ap()[lsl[l], lo:hi], P["G"][l][:, lo:hi],
                                AF.Sigmoid)
        return f

    def act_tan(l, src):
        def f(e, P):
            if src == "g":
                return e.activation(tang.ap()[lsl[l], :], P["G"][l][:, 1536:2048],
                                    AF.Tanh)
            return e.activation(tanc.ap()[lsl[l], :], c_sb.ap()[lsl[l], :], AF.Tanh)
        return f

    def act_relu(which, m):
        dst, bias_t = (out1, fc1b) if which == 1 else (out2, fc2b)
        def f(e, P):
            return e.activation(dst.ap()[:, m, :], P["ps"][m % 2], AF.Relu,
                                bias=bias_t.ap()[:, m:m + 1])
        return f

    for t in range(T):
        for l in range(L):
            w("act", "pe", f"c01_{l}@{t}")
            op("act", act_sig(l, 0, 1024), "act", f"sigif{l}@{t}")
            w("act", "pe", f"c3_{l}@{t}")
            op("act", act_tan(l, "g"), "act", f"tang{l}@{t}")
            op("act", act_sig(l, 1024, 1536), "act", f"sigo{l}@{t}")
            w("act", "dve", f"c{l}@{t}")
            op("act", act_tan(l, "c"), "act", f"tanc{l}@{t}")
        for j, g in mlp_slots(t):
            if g < 8:
                which, m = (1, g) if g < 4 else (2, g - 4)
                w("act", "pe", f"fc{which}m{m}@{j}")
                op("act", act_relu(which, m), "act", f"relu{which}m{m}@{j}")
    for g in range(8):
        j = NCHUNK - 1
        which, m = (1, g) if g < 4 else (2, g - 4)
        w("act", "pe", f"fc{which}m{m}@{j}")
        op("act", act_relu(which, m), "act", f"relu{which}m{m}@{j}")

    # ================= DVE program =================
    def dve_tt(dst, a, b, alu, l=None, dsts=None):
        def f(e, P):
            s = lsl[l] if l is not None else slice(None)
            d = dst.ap()[s, :] if dsts is None else dsts
            return e.tensor_tensor(d, a, b, alu)
        return f

    def dve_xT():
        def f(e, P):
            return e.tensor_copy(xT.ap().rearrange("p k b -> p (k b)"), P["Tp32"])
        return f

    def dve_hT(l):
        hT = h0T if l == 0 else h1T
        def f(e, P):
            return e.tensor_copy(hT.ap().rearrange("p k b -> p (k b)"), P["TpB"][l])
        return f

    def dve_copy(dst_fn):
        def f(e, P):
            d, s = dst_fn(P)
            return e.tensor_copy(d, s)
        return f

    w("dve", "pe", "xTp")
    op("dve", dve_xT(), "dve", "xT")
    for t in range(T):
        for l in range(L):
            s = lsl[l]
            if t > 0:
                w("dve", "act", f"sigif{l}@{t}")
                op("dve", dve_tt(c_sb, c_sb.ap()[s, :], sig.ap()[s, 512:1024],
                                 MUL, l=l))
            w("dve", "act", f"tang{l}@{t}")
            op("dve", dve_tt(tmp, sig.ap()[s, 0:512], tang.ap()[s, :], MUL, l=l))
            if t == 0:
                op("dve", dve_copy(lambda P, s=s: (c_sb.ap()[s, :], tmp.ap()[s, :])),
                   "dve", f"c{l}@{t}")
            else:
                op("dve", dve_tt(c_sb, c_sb.ap()[s, :], tmp.ap()[s, :], ADD, l=l),
                   "dve", f"c{l}@{t}")
            w("dve", "act", f"tanc{l}@{t}")
            op("dve", dve_tt(hnew, sig.ap()[s, 1024:1536], tanc.ap()[s, :],
                             MUL, l=l), "dve", f"h{l}@{t}")
            w("dve", "pe", f"T{l}@{t}")
            op("dve", dve_hT(l), "dve", f"hT{l}@{t}")
        op("dve", dve_tt(hsumT, h0T.ap(), h1T.ap(), ADD, dsts=hsumT.ap()))
        blk = t // RING
        if blk >= 2:
            w("dve", "pe", f"fc1m3@{blk - 2}")
        op("dve", dve_copy(lambda P, r=blk % 2, sl=t % RING:
                           (ring[r].ap()[:, :, sl, :], hsumT.ap())),
           "dve", f"hsum@{t}")
        for j, g in mlp_slots(t):
            if g >= 8:
                m = g - 8
                w("dve", "pe", f"fc3m{m}@{j}")
                if j > 0:
                    w("dve", "dout", f"out{m}@{j - 1}")
                op("dve", dve_copy(lambda P, m=m: (out3[m].ap(), P["ps"][m % 2])),
                   "dve", f"o3m{m}@{j}")
    for m in range(4):
        j = NCHUNK - 1
        w("dve", "pe", f"fc3m{m}@{j}")
        w("dve", "dout", f"out{m}@{j - 1}")
        op("dve", dve_copy(lambda P, m=m: (out3[m].ap(), P["ps"][m % 2])),
           "dve", f"o3m{m}@{j}")

    # ================= SYNC (DMA) program =================
    def s_load(dst, src, sem):
        def f(e, P):
            return e.dma_start(out=dst, in_=src)
        return (f, sem)

    loads = [
        s_load(x_sb.ap(), x_d.ap(), "lstm_in"),
        s_load(wih[0].ap(), wih_d[0].ap().rearrange("k p c -> p k c"), "lstm_in"),
        s_load(whh[0].ap(), whh_d[0].ap().rearrange("k p c -> p k c"), "lstm_in"),
        s_load(biasr[0].ap(), bias_d[0].ap(), "lstm_in"),
        s_load(wih[1].ap(), wih_d[1].ap().rearrange("k p c -> p k c"), "lstm_in"),
        s_load(whh[1].ap(), whh_d[1].ap().rearrange("k p c -> p k c"), "lstm_in"),
        s_load(biasr[1].ap(), bias_d[1].ap(), "lstm_in"),
        s_load(id64.ap(), id64_d.ap(), "lstm_in"),
        s_load(id64b.ap(), id64b_d.ap(), "lstm_in"),
        s_load(ones64.ap(), ones64_d.ap(), "lstm_in"),
        s_load(fc1w.ap(), fc1w_d.ap().rearrange("k p c -> p k c"), "mlp_in"),
        s_load(fc2w.ap(), fc2w_d.ap().rearrange("k p c -> p k c"), "mlp_in"),
        s_load(fc3w.ap(), fc3w_d.ap().rearrange("k p c -> p k c"), "mlp_in"),
        s_load(fc1b.ap(), fc1b_d.ap(), "mlp_in"),
        s_load(fc2b.ap(), fc2b_d.ap(), "mlp_in"),
        s_load(fc3br.ap(), fc3b_d.ap(), "mlp_in"),
        s_load(ones128.ap(), ones128_d.ap(), "mlp_in"),
    ]
    assert sum(1 for _, s in loads if s == "lstm_in") == N_LSTM_LOAD
    assert sum(1 for _, s in loads if s == "mlp_in") == N_MLP_LOAD

    def s_out(j, m):
        tt = 8 * j + 2 * m
        def f(e, P):
            return e.dma_start(out=out_d.ap()[:, tt:tt + 2, :]
                               .rearrange("b u h -> u b h"), in_=out3[m].ap())
        return f

    for j in range(NCHUNK):
        for m in range(4):
            w("sync", "dve", f"o3m{m}@{j}")
            op("sync", s_out(j, m), "dout", f"out{m}@{j}", n=16)

    # ================= emission =================
    with (
        nc.psum_tensor("P", [128, 4096], F32) as P_,
        nc.semaphore("lstm_in") as lstm_in,
        nc.semaphore("mlp_in") as mlp_in,
        nc.semaphore("dma_out") as dma_out,
        nc.semaphore("pe_s") as pe_s,
        nc.semaphore("act_s") as act_s,
        nc.semaphore("dve_s") as dve_s,
        nc.Block() as block,
    ):
        Pap = P_.ap()
        P = {
            "G": [Pap[0:64, 0:2048], Pap[64:128, 0:2048]],
            "ps": [Pap[:, 2048:2560], Pap[:, 2560:3072]],
            "Tp32": Pap[0:128, 3072:3328],
            "TpB": [Pap[0:128, 3328 + 128 * i:3456 + 128 * i].bitcast(BF16)
                    for i in range(2)],
        }
        sems = {"pe": pe_s, "act": act_s, "dve": dve_s, "dout": dma_out,
                "lstm_in": lstm_in, "mlp_in": mlp_in}

        def emit(e, prog):
            for item in prog:
                if item[0] == "w":
                    _, sem, key = item
                    v = key if isinstance(key, int) else val[sem][key]
                    e.wait_ge(sems[sem], v)
                else:
                    _, fn, sem, n = item
                    inst = fn(e, P)
                    if sem is not None:
                        inst.then_inc(sems[sem], n)

        @block.sync
        def _(sync):
            for fn, sem in loads:
                fn(sync, P).then_inc(sems[sem], 16)
            emit(sync, progs["sync"])

        @block.tensor
        def _(tensor):
            emit(tensor, progs["pe"])

        @block.scalar
        def _(scalar):
            emit(scalar, progs["act"])

        @block.vector
        def _(vector):
            emit(vector, progs["dve"])

    return nc


_PERM = None


def _gate_perm():
    # torch gate order (i, f, g, o) -> our column order (i, f, o, g)
    global _PERM
    if _PERM is None:
        i = np.arange(512)
        _PERM = np.concatenate([i, 512 + i, 1536 + i, 1024 + i])
    return _PERM


def _prep_inputs(x, W_ih, W_hh, b_ih, b_hh, fc1_w, fc1_b, fc2_w, fc2_b, fc3_w, fc3_b):
    perm = _gate_perm()
    bf = ml_dtypes.bfloat16
    common = {}
    for l in range(L):
        wt = np.ascontiguousarray(W_ih[l][perm].T)          # [512, 2048]
        common[f"wih{l}"] = wt.reshape(KT, 128, G).astype(bf)
        wt = np.ascontiguousarray(W_hh[l][perm].T)
        common[f"whh{l}"] = wt.reshape(KT, 128, G).astype(bf)
        common[f"bias{l}"] = (b_ih[l] + b_hh[l])[perm].reshape(1, G).astype(bf)
    common["fc1w"] = np.ascontiguousarray(fc1_w.T).reshape(KT, 128, 512).astype(bf)
    common["fc2w"] = np.ascontiguousarray(fc2_w.T).reshape(KT, 128, 512).astype(bf)
    common["fc3w"] = np.ascontiguousarray(fc3_w.T).reshape(KT, 128, 512).astype(bf)
    common["fc1b"] = np.ascontiguousarray(fc1_b.reshape(4, 128).T)
    common["fc2b"] = np.ascontiguousarray(fc2_b.reshape(4, 128).T)
    common["fc3b"] = fc3_b.reshape(1, 512).astype(bf)
    common["ones64"] = np.ones((1, 64), bf)
    common["ones128"] = np.ones((1, 128), bf)
    common["id64"] = np.eye(64, dtype=np.float32)
    eye = np.eye(64)
    common["id64b"] = np.concatenate([eye, eye], axis=0).astype(bf)
    in_maps = []
    for c in range(NCORES):
        m = dict(common)
        m["x"] = np.ascontiguousarray(x[BC * c:BC * (c + 1)])
        in_maps.append(m)
    return in_maps


_NC_CACHE = None


def kernel(**inputs):
    global _NC_CACHE
    if _NC_CACHE is None:
        _NC_CACHE = build_nc()
    nc = _NC_CACHE
    in_maps = _prep_inputs(**{k: np.asarray(v) for k, v in inputs.items()})
    res = run_bass_kernel_spmd(nc, in_maps, core_ids=list(range(NCORES)))
    out = np.concatenate([res.results[c]["out"] for c in range(NCORES)], axis=0)
    return out.astype(np.float32)
